# revision 1
# baseline (speedup 1.0000x reference)
"""MoE FeedForward (top-2 of 4 experts) — expert-parallel Trainium2 kernel.

Strategy (matches the sharding hint): the tiny gating matmul + top-k routing
run on host as part of input sharding; tokens are dispatched by gate index to
expert-owning cores (expert e -> cores 2e, 2e+1, each taking half of that
expert's tokens, padded to a common capacity C). Each core computes
    y^T = (relu(W1^T @ x^T + b1) -> W2^T @ mid + b2) * gate
entirely in transposed [feature, token] layout (no on-device transposes),
with bf16 matmuls accumulating in fp32 PSUM. The host combine scatter-adds
the two gate-weighted expert contributions per token.

Model dims (hardcoded per problem spec): N=8192 tokens, D=512, H=2048,
E=4 experts, top-k=2, 8 NeuronCores.
"""

import numpy as np
import ml_dtypes
from contextlib import ExitStack

D = 512
H = 2048
E = 4
TOP_K = 2
N_CORES = 8
ND = D // 128   # 4 d-tiles
NH = H // 128   # 16 h-tiles

_NC_CACHE = {}


def _build_moe_nc(C: int, chunk: int = 512):
    """Per-core SPMD program: [D,C] bf16 tokens -> [D,C] f32 gated output."""
    import concourse.mybir as mybir
    from concourse import bacc, tile

    dt = mybir.dt
    AF = mybir.ActivationFunctionType

    assert C % 128 == 0
    chunks = []
    off = 0
    while off < C:
        s = min(chunk, C - off)
        chunks.append((off, s))
        off += s

    nc = bacc.Bacc(None, target_bir_lowering=False)
    xt = nc.dram_tensor("xt", [D, C], dt.bfloat16, kind="ExternalInput")
    w1 = nc.dram_tensor("w1", [D, H], dt.bfloat16, kind="ExternalInput")
    w2 = nc.dram_tensor("w2", [H, D], dt.bfloat16, kind="ExternalInput")
    b1r = nc.dram_tensor("b1r", [128, NH], dt.float32, kind="ExternalInput")
    b2r = nc.dram_tensor("b2r", [128, ND], dt.float32, kind="ExternalInput")
    gr = nc.dram_tensor("gr", [128, C], dt.float32, kind="ExternalInput")
    yt = nc.dram_tensor("yt", [D, C], dt.float32, kind="ExternalOutput")

    with tile.TileContext(nc) as tc, ExitStack() as ctx:
        wpool = ctx.enter_context(tc.tile_pool(name="weights", bufs=1))
        xpool = ctx.enter_context(tc.tile_pool(name="x", bufs=1))
        midp = ctx.enter_context(tc.tile_pool(name="mid", bufs=18))
        p1 = ctx.enter_context(tc.tile_pool(name="p1", bufs=3, space="PSUM"))
        p2 = ctx.enter_context(tc.tile_pool(name="p2", bufs=3, space="PSUM"))
        ypool = ctx.enter_context(tc.tile_pool(name="y", bufs=4))

        # persistent loads: order = what the PE needs first
        w1_sb = []
        for i in range(ND):
            t = wpool.tile([128, H], dt.bfloat16, tag=f"w1_{i}", name=f"w1_{i}")
            nc.sync.dma_start(t[:], w1[i * 128:(i + 1) * 128, :])
            w1_sb.append(t)
        xt_sb = []
        for i in range(ND):
            t = xpool.tile([128, C], dt.bfloat16, tag=f"xt_{i}", name=f"xt_{i}")
            nc.sync.dma_start(t[:], xt[i * 128:(i + 1) * 128, :])
            xt_sb.append(t)
        b1_sb = wpool.tile([128, NH], dt.float32, tag="b1", name="b1_sb")
        nc.sync.dma_start(b1_sb[:], b1r[:])
        w2_sb = []
        for i in range(NH):
            t = wpool.tile([128, D], dt.bfloat16, tag=f"w2_{i}", name=f"w2_{i}")
            nc.sync.dma_start(t[:], w2[i * 128:(i + 1) * 128, :])
            w2_sb.append(t)
        b2_sb = wpool.tile([128, ND], dt.float32, tag="b2", name="b2_sb")
        nc.sync.dma_start(b2_sb[:], b2r[:])
        gr_sb = wpool.tile([128, C], dt.float32, tag="gr", name="gr_sb")
        nc.sync.dma_start(gr_sb[:], gr[:])

        for (c0, S) in chunks:
            # GEMM1: mid^T[h, c] = relu(sum_d w1[d,h] * x^T[d,c] + b1[h])
            mids = []
            for ht in range(NH):
                ps = p1.tile([128, S], dt.float32, tag="ps1", name=f"ps1_{c0}_{ht}")
                for di in range(ND):
                    nc.tensor.matmul(
                        ps[:],
                        w1_sb[di][:, ht * 128:(ht + 1) * 128],
                        xt_sb[di][:, c0:c0 + S],
                        start=(di == 0),
                        stop=(di == ND - 1),
                    )
                m = midp.tile([128, S], dt.bfloat16, tag="mid", name=f"mid_{c0}_{ht}")
                nc.scalar.activation(m[:], ps[:], AF.Relu, bias=b1_sb[:, ht:ht + 1])
                mids.append(m)
            # GEMM2: y^T[d, c] = (sum_h w2[h,d] * mid^T[h,c] + b2[d]) * g[c]
            for di in range(ND):
                ps2 = p2.tile([128, S], dt.float32, tag="ps2", name=f"ps2_{c0}_{di}")
                for ht in range(NH):
                    nc.tensor.matmul(
                        ps2[:],
                        w2_sb[ht][:, di * 128:(di + 1) * 128],
                        mids[ht][:],
                        start=(ht == 0),
                        stop=(ht == NH - 1),
                    )
                yt_t = ypool.tile([128, S], dt.float32, tag="y", name=f"y_{c0}_{di}")
                nc.scalar.activation(yt_t[:], ps2[:], AF.Identity, bias=b2_sb[:, di:di + 1])
                nc.vector.tensor_mul(yt_t[:], yt_t[:], gr_sb[:, c0:c0 + S])
                nc.sync.dma_start(yt[di * 128:(di + 1) * 128, c0:c0 + S], yt_t[:])

    nc.finalize()
    return nc


def _route(h, w_gate):
    """Top-2 gating, matching jax.lax.top_k (ties -> lower index) + softmax."""
    logits = h @ w_gate                                      # [N, E] f32
    order = np.argsort(-logits, axis=1, kind="stable")
    top_idx = order[:, :TOP_K]                               # [N, 2]
    top_lg = np.take_along_axis(logits, top_idx, axis=1)
    mx = top_lg.max(axis=1, keepdims=True)
    ex = np.exp(top_lg - mx)
    gates2 = (ex / ex.sum(axis=1, keepdims=True)).astype(np.float32)
    return top_idx, gates2


def _run(inputs, trace=False):
    from concourse.bass_utils import run_bass_kernel_spmd

    bf16 = ml_dtypes.bfloat16
    h = np.asarray(inputs["h"], dtype=np.float32)
    w_gate = np.asarray(inputs["w_gate"], dtype=np.float32)
    w1 = np.asarray(inputs["w1"], dtype=np.float32)
    b1 = np.asarray(inputs["b1"], dtype=np.float32)
    w2 = np.asarray(inputs["w2"], dtype=np.float32)
    b2 = np.asarray(inputs["b2"], dtype=np.float32)
    N = h.shape[0]

    top_idx, gates2 = _route(h, w_gate)

    # dispatch: expert e -> cores 2e (first half) and 2e+1 (second half)
    core_toks, core_gates, core_expert = [], [], []
    for e in range(E):
        sel = top_idx == e                                   # [N, 2] bool
        toks = np.nonzero(sel.any(axis=1))[0]
        g = gates2[toks, sel[toks].argmax(axis=1)]
        half = (len(toks) + 1) // 2
        for lo, hi in ((0, half), (half, len(toks))):
            core_toks.append(toks[lo:hi])
            core_gates.append(g[lo:hi])
            core_expert.append(e)

    maxlen = max(len(t) for t in core_toks)
    C = max(128, -(-maxlen // 128) * 128)

    if C not in _NC_CACHE:
        _NC_CACHE[C] = _build_moe_nc(C)
    nc = _NC_CACHE[C]

    in_maps = []
    for c in range(N_CORES):
        e = core_expert[c]
        toks = core_toks[c]
        n = len(toks)
        xt = np.zeros((D, C), dtype=bf16)
        xt[:, :n] = h[toks].T.astype(bf16)
        grow = np.zeros(C, dtype=np.float32)
        grow[:n] = core_gates[c]
        in_maps.append({
            "xt": xt,
            "w1": w1[e].astype(bf16),
            "w2": w2[e].astype(bf16),
            "b1r": np.ascontiguousarray(b1[e].reshape(NH, 128).T),
            "b2r": np.ascontiguousarray(b2[e].reshape(ND, 128).T),
            "gr": np.ascontiguousarray(np.broadcast_to(grow, (128, C))),
        })

    res = run_bass_kernel_spmd(nc, in_maps, core_ids=list(range(N_CORES)),
                               trace=trace)

    out = np.zeros((N, D), dtype=np.float32)
    for c in range(N_CORES):
        toks = core_toks[c]
        if len(toks):
            out[toks] += res.results[c]["yt"][:, :len(toks)].T
    return out, res


def kernel(**inputs) -> np.ndarray:
    out, _ = _run(inputs, trace=False)
    return out
